# revision 3
# baseline (speedup 1.0000x reference)
"""Multi-head attention (B=4, L=2048, D=512, H=8) on 8 Trainium2 cores.

Sharding: core c handles batch b = c//2, query rows [(c%2)*1024, +1024).
Each core projects its batch's full K/V (2x redundant across the pair of
cores sharing a batch) so attention is fully local -- no collectives.

Device layouts (per core):
  xqT (512, 1024) bf16   query slice, transposed (dmodel on partitions)
  xkT/xvT (512, 2048)    full batch key/value, transposed
  qT_all / kT_all        projections kept transposed: head h lives in
                         dmodel-chunk tile h//2 at partition offset 64*(h%2)
  V_sb (128, 520) x16    V natural layout per kv chunk; head h at cols
                         [65h, 65h+64), col 65h+64 = ones (softmax denom)
  scoresT (128kv, 1024q) PSUM; exp+mask+scale fused into one ACT op
  xsT_ext (65, 512) PSUM row 64 = softmax denominator
"""
import numpy as np
import ml_dtypes

import concourse.bacc as bacc
import concourse.bass as bass
import concourse.mybir as mybir
import concourse.tile as tile
from concourse.bass_utils import run_bass_kernel_spmd

F32 = mybir.dt.float32
BF16 = mybir.dt.bfloat16
AF = mybir.ActivationFunctionType

B, L, D = 4, 2048, 512
H, DK = 8, 64
N_CORES = 8
LQ = L // 2            # query rows per core
P = 128
KVC = L // P           # 16 kv chunks
QT = LQ // P           # 8 query tiles of 128
MC = D // P            # 4 dmodel chunks
MASK_BIAS = np.float32(-1e30)

# matmul operand dtype knob: BF16 (fast, ~1e-2 err) or F32R (fp32 storage,
# relaxed-precision matmul at the same PE rate for N>=256)
MM_DT = BF16
MM_NP = ml_dtypes.bfloat16 if MM_DT == BF16 else np.float32

_cache = {}


def _build():
    nc = bacc.Bacc("TRN2", target_bir_lowering=False, debug=False)

    xqT_d = nc.dram_tensor("xqT", [D, LQ], MM_DT, kind="ExternalInput").ap()
    xkT_d = nc.dram_tensor("xkT", [D, L], MM_DT, kind="ExternalInput").ap()
    xvT_d = nc.dram_tensor("xvT", [D, L], MM_DT, kind="ExternalInput").ap()
    wq_d = nc.dram_tensor("wq", [D, D], MM_DT, kind="ExternalInput").ap()
    wk_d = nc.dram_tensor("wk", [D, D], MM_DT, kind="ExternalInput").ap()
    wv_d = nc.dram_tensor("wv", [D, D], MM_DT, kind="ExternalInput").ap()
    wo_d = nc.dram_tensor("wo", [D, D], MM_DT, kind="ExternalInput").ap()
    bq_d = nc.dram_tensor("bq", [P, MC], F32, kind="ExternalInput").ap()
    bk_d = nc.dram_tensor("bk", [P, MC], F32, kind="ExternalInput").ap()
    bv_d = nc.dram_tensor("bv", [1, D], MM_DT, kind="ExternalInput").ap()
    bo_d = nc.dram_tensor("bo", [1, D], F32, kind="ExternalInput").ap()
    mb_d = nc.dram_tensor("mb", [P, KVC], F32, kind="ExternalInput").ap()
    out_d = nc.dram_tensor("out", [LQ, D], F32, kind="ExternalOutput").ap()

    with tile.TileContext(nc) as tc:
        with tc.tile_pool(name="const", bufs=1) as cpool, \
             tc.tile_pool(name="xin", bufs=1) as xpool, \
             tc.tile_pool(name="proj", bufs=1) as prpool, \
             tc.tile_pool(name="attn", bufs=17) as apool, \
             tc.tile_pool(name="norm", bufs=4) as npool, \
             tc.tile_pool(name="outp", bufs=3) as opool, \
             tc.tile_pool(name="ps", bufs=2, space="PSUM") as ps:

            # ---- constants / weights ----
            wq = cpool.tile_from(wq_d.rearrange("(kc p) n -> p kc n", p=P))
            wk = cpool.tile_from(wk_d.rearrange("(kc p) n -> p kc n", p=P))
            wv = cpool.tile_from(wv_d.rearrange("(kc p) n -> p kc n", p=P))
            wo = cpool.tile_from(wo_d.rearrange("(kc p) n -> p kc n", p=P))
            bq = cpool.tile_from(bq_d)
            bk = cpool.tile_from(bk_d)
            bv = cpool.tile_from(bv_d)
            bo = cpool.tile_from(bo_d)
            mb = cpool.tile_from(mb_d)
            ones1 = cpool.tile([1, P], MM_DT)
            nc.vector.memset(ones1[:], 1.0)
            bo_bc = cpool.tile([P, D], F32)
            nc.gpsimd.partition_broadcast(bo_bc[:], bo[:])

            xqT = xpool.tile_from(xqT_d.rearrange("(kc p) n -> p kc n", p=P))
            xkT = xpool.tile_from(xkT_d.rearrange("(kc p) n -> p kc n", p=P))
            xvT = xpool.tile_from(xvT_d.rearrange("(kc p) n -> p kc n", p=P))

            # ---- Q/K projections (transposed outputs) ----
            qT = [prpool.tile([P, LQ], MM_DT, tag=f"qT{m}", name=f"qT{m}") for m in range(MC)]
            kT = [prpool.tile([P, L], MM_DT, tag=f"kT{m}", name=f"kT{m}") for m in range(MC)]
            for m in range(MC):
                for s in range(LQ // 512):
                    pp = ps.tile([P, 512], F32, tag="proj")
                    for kc in range(MC):
                        nc.tensor.matmul(
                            pp[:], wq[:, kc, m * P:(m + 1) * P],
                            xqT[:, kc, s * 512:(s + 1) * 512],
                            start=kc == 0, stop=kc == MC - 1)
                    nc.scalar.activation(qT[m][:, s * 512:(s + 1) * 512], pp[:],
                                         AF.Identity, bias=bq[:, m:m + 1])
                for s in range(L // 512):
                    pp = ps.tile([P, 512], F32, tag="proj")
                    for kc in range(MC):
                        nc.tensor.matmul(
                            pp[:], wk[:, kc, m * P:(m + 1) * P],
                            xkT[:, kc, s * 512:(s + 1) * 512],
                            start=kc == 0, stop=kc == MC - 1)
                    nc.scalar.activation(kT[m][:, s * 512:(s + 1) * 512], pp[:],
                                         AF.Identity, bias=bk[:, m:m + 1])

            # ---- V projection (natural layout + ones column per head) ----
            V = [prpool.tile([P, H * 65], MM_DT, tag=f"V{t}", name=f"V{t}") for t in range(KVC)]
            for t in range(KVC):
                pv = ps.tile([P, D], F32, tag="proj")
                for kc in range(MC):
                    nc.tensor.matmul(pv[:], xvT[:, kc, t * P:(t + 1) * P],
                                     wv[:, kc, :], start=kc == 0, stop=False)
                nc.tensor.matmul(pv[:], ones1[0:1, :], bv[0:1, :],
                                 start=False, stop=True)
                vv = V[t].rearrange("p (g d) -> p g d", d=65)
                nc.vector.tensor_copy(vv[:, :, 0:64],
                                      pv.rearrange("p (g d) -> p g d", d=64))
                nc.vector.memset(vv[:, :, 64:65], 1.0)

            # ---- flash attention per head ----
            xsT2 = [prpool.tile([P, LQ], MM_DT, tag=f"xs{hp}", name=f"xsT2_{hp}") for hp in range(MC)]
            for h in range(H):
                hp, po = h // 2, 64 * (h % 2)
                at = []
                for c in range(KVC):
                    ss = ps.tile([P, 1024], F32, tag="scores")
                    for qh in range(2):
                        nc.tensor.matmul(
                            ss[:, qh * 512:(qh + 1) * 512],
                            kT[hp][po:po + 64, c * P:(c + 1) * P],
                            qT[hp][po:po + 64, qh * 512:(qh + 1) * 512],
                            start=True, stop=True)
                    a = apool.tile([P, 1024], MM_DT, tag="at")
                    nc.scalar.activation(a[:], ss[:], AF.Exp,
                                         bias=mb[:, c:c + 1], scale=0.125)
                    at.append(a)
                xs = [ps.tile([65, 512], F32, tag="xs", name=f"xs_h{h}_{qh}") for qh in range(2)]
                for c in range(KVC):
                    for qh in range(2):
                        nc.tensor.matmul(
                            xs[qh][:], V[c][:, 65 * h:65 * h + 65],
                            at[c][:, qh * 512:(qh + 1) * 512],
                            start=c == 0, stop=c == KVC - 1)
                for qh in range(2):
                    rec = npool.tile([1, 512], F32, tag="rec")
                    nc.vector.reciprocal(rec[:], xs[qh][64:65, :])
                    bc = npool.tile([64, 512], F32, tag="bc")
                    nc.gpsimd.partition_broadcast(bc[:], rec[:])
                    nc.vector.tensor_mul(
                        xsT2[hp][po:po + 64, qh * 512:(qh + 1) * 512],
                        xs[qh][0:64, :], bc[:])

            # ---- output projection ----
            for qt in range(QT):
                po_ = ps.tile([P, D], F32, tag="proj")
                for hp in range(MC):
                    nc.tensor.matmul(po_[:], xsT2[hp][:, qt * P:(qt + 1) * P],
                                     wo[:, hp, :], start=hp == 0, stop=hp == MC - 1)
                osb = opool.tile([P, D], F32, tag="osb")
                nc.vector.tensor_add(osb[:], po_[:], bo_bc[:])
                nc.sync.dma_start(out_d[qt * P:(qt + 1) * P, :], osb[:])

    nc.compile()
    return nc


def _host_inputs(query, key, value, mask, Wq, bq, Wk, bk, Wv, bv, Wo, bo):
    """Build the 8 per-core input maps (all rank-dependence lives here)."""
    f32 = np.float32
    wq_ = np.ascontiguousarray(Wq).astype(MM_NP)
    wk_ = np.ascontiguousarray(Wk).astype(MM_NP)
    wv_ = np.ascontiguousarray(Wv).astype(MM_NP)
    wo_ = np.ascontiguousarray(Wo).astype(MM_NP)
    bq_ = np.ascontiguousarray(bq.astype(f32).reshape(MC, P).T)
    bk_ = np.ascontiguousarray(bk.astype(f32).reshape(MC, P).T)
    bv_ = bv.astype(MM_NP).reshape(1, D)
    bo_ = bo.astype(f32).reshape(1, D)
    in_maps = []
    for c in range(N_CORES):
        b, half = c // 2, c % 2
        xqT = np.ascontiguousarray(
            query[b, half * LQ:(half + 1) * LQ, :].T).astype(MM_NP)
        xkT = np.ascontiguousarray(key[b].T).astype(MM_NP)
        xvT = np.ascontiguousarray(value[b].T).astype(MM_NP)
        mbias = np.where(mask[b] == 0, MASK_BIAS, f32(0.0)).astype(f32)
        mb_ = np.ascontiguousarray(mbias.reshape(KVC, P).T)
        in_maps.append({
            "xqT": xqT, "xkT": xkT, "xvT": xvT,
            "wq": wq_, "wk": wk_, "wv": wv_, "wo": wo_,
            "bq": bq_, "bk": bk_, "bv": bv_, "bo": bo_, "mb": mb_,
        })
    return in_maps


def kernel(query, key, value, mask, Wq, bq, Wk, bk, Wv, bv, Wo, bo):
    if "nc" not in _cache:
        _cache["nc"] = _build()
    nc = _cache["nc"]
    in_maps = _host_inputs(query, key, value, mask,
                           Wq, bq, Wk, bk, Wv, bv, Wo, bo)
    res = run_bass_kernel_spmd(nc, in_maps, list(range(N_CORES))).results
    out = np.empty((B, L, D), np.float32)
    for c in range(N_CORES):
        b, half = c // 2, c % 2
        out[b, half * LQ:(half + 1) * LQ, :] = res[c]["out"]
    return out


# revision 7
# speedup vs baseline: 1.4420x; 1.4420x over previous
"""Multi-head attention (B=4, L=2048, D=512, H=8) on 8 Trainium2 cores.

Sharding: core c handles batch b = c//2, query rows [(c%2)*1024, +1024).
Each core projects its batch's full K/V (2x redundant across the pair of
cores sharing a batch) so attention is fully local -- no collectives.

Device layouts (per core):
  xqT (512, 1024) bf16   query slice, transposed (dmodel on partitions)
  xkT/xvT (512, 2048)    full batch key/value, transposed
  qT_all / kT_all        projections kept transposed: head h lives in
                         dmodel-chunk tile h//2 at partition offset 64*(h%2)
  V_sb (128, 520) x16    V natural layout per kv chunk; head h at cols
                         [65h, 65h+64), col 65h+64 = ones (softmax denom)
  scoresT (128kv, 1024q) PSUM; exp+mask+scale fused into one ACT op
  xsT_ext (65, 512) PSUM row 64 = softmax denominator
"""
import numpy as np
import ml_dtypes

import concourse.bacc as bacc
import concourse.bass as bass
import concourse.mybir as mybir
import concourse.tile as tile
from concourse.bass_utils import run_bass_kernel_spmd

F32 = mybir.dt.float32
BF16 = mybir.dt.bfloat16
AF = mybir.ActivationFunctionType

B, L, D = 4, 2048, 512
H, DK = 8, 64
N_CORES = 8
LQ = L // 2            # query rows per core
P = 128
KVC = L // P           # 16 kv chunks
QT = LQ // P           # 8 query tiles of 128
MC = D // P            # 4 dmodel chunks
MASK_BIAS = np.float32(-1e30)

# matmul operand dtype knob: BF16 (fast, ~1e-2 err) or F32R (fp32 storage,
# relaxed-precision matmul at the same PE rate for N>=256)
MM_DT = BF16
MM_NP = ml_dtypes.bfloat16 if MM_DT == BF16 else np.float32

_cache = {}


def _build():
    nc = bacc.Bacc("TRN2", target_bir_lowering=False, debug=False)

    xqT_d = nc.dram_tensor("xqT", [D, LQ], MM_DT, kind="ExternalInput").ap()
    xkT_d = nc.dram_tensor("xkT", [D, L], MM_DT, kind="ExternalInput").ap()
    xvT_d = nc.dram_tensor("xvT", [D, L], MM_DT, kind="ExternalInput").ap()
    wq_d = nc.dram_tensor("wq", [D, D], MM_DT, kind="ExternalInput").ap()
    wk_d = nc.dram_tensor("wk", [D, D], MM_DT, kind="ExternalInput").ap()
    wv_d = nc.dram_tensor("wv", [D, D], MM_DT, kind="ExternalInput").ap()
    wo_d = nc.dram_tensor("wo", [D, D], MM_DT, kind="ExternalInput").ap()
    bq_d = nc.dram_tensor("bq", [P, MC], F32, kind="ExternalInput").ap()
    bk_d = nc.dram_tensor("bk", [P, MC], F32, kind="ExternalInput").ap()
    bv_d = nc.dram_tensor("bv", [1, D], MM_DT, kind="ExternalInput").ap()
    bo_d = nc.dram_tensor("bo", [1, D], F32, kind="ExternalInput").ap()
    mb_d = nc.dram_tensor("mb", [P, KVC], F32, kind="ExternalInput").ap()
    out_d = nc.dram_tensor("out", [LQ, D], F32, kind="ExternalOutput").ap()

    with tile.TileContext(nc) as tc:
        with tc.tile_pool(name="const", bufs=1) as cpool, \
             tc.tile_pool(name="xin", bufs=1) as xpool, \
             tc.tile_pool(name="proj", bufs=1) as prpool, \
             tc.tile_pool(name="attn", bufs=17) as apool, \
             tc.tile_pool(name="norm", bufs=4) as npool, \
             tc.tile_pool(name="outp", bufs=3) as opool, \
             tc.tile_pool(name="ps", bufs=2, space="PSUM") as ps:

            # ---- constants / weights (first-needed first for DMA order) ----
            wq = cpool.tile_from(wq_d.rearrange("(kc p) n -> p kc n", p=P))
            xqT = xpool.tile_from(xqT_d.rearrange("(kc p) n -> p kc n", p=P))
            bq = cpool.tile_from(bq_d)
            wk = cpool.tile_from(wk_d.rearrange("(kc p) n -> p kc n", p=P))
            xkT = xpool.tile_from(xkT_d.rearrange("(kc p) n -> p kc n", p=P))
            bk = cpool.tile_from(bk_d)
            wv = cpool.tile_from(wv_d.rearrange("(kc p) n -> p kc n", p=P))
            xvT = xpool.tile_from(xvT_d.rearrange("(kc p) n -> p kc n", p=P))
            wo = cpool.tile_from(wo_d.rearrange("(kc p) n -> p kc n", p=P))
            bv = cpool.tile_from(bv_d)
            bo = cpool.tile_from(bo_d)
            mb = cpool.tile_from(mb_d)
            ones1 = cpool.tile([1, P], MM_DT)
            nc.vector.memset(ones1[:], 1.0)
            bo_bc = cpool.tile([P, D], F32)
            nc.gpsimd.partition_broadcast(bo_bc[:], bo[:])

            # ---- Q/K projections (transposed outputs) ----
            qT = [prpool.tile([P, LQ], MM_DT, tag=f"qT{m}", name=f"qT{m}") for m in range(MC)]
            kT = [prpool.tile([P, L], MM_DT, tag=f"kT{m}", name=f"kT{m}") for m in range(MC)]
            for m in range(MC):
                for s in range(LQ // 512):
                    pp = ps.tile([P, 512], F32, tag="proj")
                    for kc in range(MC):
                        nc.tensor.matmul(
                            pp[:], wq[:, kc, m * P:(m + 1) * P],
                            xqT[:, kc, s * 512:(s + 1) * 512],
                            start=kc == 0, stop=kc == MC - 1)
                    nc.vector.tensor_scalar_add(qT[m][:, s * 512:(s + 1) * 512],
                                                pp[:], bq[:, m:m + 1])
                for s in range(L // 512):
                    pp = ps.tile([P, 512], F32, tag="proj")
                    for kc in range(MC):
                        nc.tensor.matmul(
                            pp[:], wk[:, kc, m * P:(m + 1) * P],
                            xkT[:, kc, s * 512:(s + 1) * 512],
                            start=kc == 0, stop=kc == MC - 1)
                    nc.vector.tensor_scalar_add(kT[m][:, s * 512:(s + 1) * 512],
                                                pp[:], bk[:, m:m + 1])

            # ---- V projection (natural layout + ones column per head) ----
            V = [prpool.tile([P, H * 65], MM_DT, tag=f"V{t}", name=f"V{t}") for t in range(KVC)]
            for t in range(KVC):
                pv = ps.tile([P, D], F32, tag="proj")
                for kc in range(MC):
                    nc.tensor.matmul(pv[:], xvT[:, kc, t * P:(t + 1) * P],
                                     wv[:, kc, :], start=kc == 0, stop=False)
                nc.tensor.matmul(pv[:], ones1[0:1, :], bv[0:1, :],
                                 start=False, stop=True)
                vv = V[t].rearrange("p (g d) -> p g d", d=65)
                nc.vector.tensor_copy(vv[:, :, 0:64],
                                      pv.rearrange("p (g d) -> p g d", d=64))
                nc.vector.memset(vv[:, :, 64:65], 1.0)

            # ---- flash attention per head ----
            xsT2 = [prpool.tile([P, LQ], MM_DT, tag=f"xs{hp}", name=f"xsT2_{hp}") for hp in range(MC)]
            for h in range(H):
                hp, po = h // 2, 64 * (h % 2)
                at = []
                for c in range(KVC):
                    ss = ps.tile([P, 1024], F32, tag="scores")
                    for qh in range(2):
                        nc.tensor.matmul(
                            ss[:, qh * 512:(qh + 1) * 512],
                            kT[hp][po:po + 64, c * P:(c + 1) * P],
                            qT[hp][po:po + 64, qh * 512:(qh + 1) * 512],
                            start=True, stop=True)
                    a = apool.tile([P, 1024], MM_DT, tag="at")
                    nc.scalar.activation(a[:], ss[:], AF.Exp,
                                         bias=mb[:, c:c + 1], scale=0.125)
                    at.append(a)
                xs = [ps.tile([65, 512], F32, tag="xs", name=f"xs_h{h}_{qh}") for qh in range(2)]
                for c in range(KVC):
                    for qh in range(2):
                        nc.tensor.matmul(
                            xs[qh][:], V[c][:, 65 * h:65 * h + 65],
                            at[c][:, qh * 512:(qh + 1) * 512],
                            start=c == 0, stop=c == KVC - 1)
                for qh in range(2):
                    srow = npool.tile([1, 512], F32, tag="srow")
                    nc.vector.tensor_copy(srow[:], xs[qh][64:65, :])
                    rec = npool.tile([1, 512], F32, tag="rec")
                    nc.vector.reciprocal_approx_fast(rec[:], srow[:])
                    bc = npool.tile([64, 512], F32, tag="bc")
                    nc.gpsimd.partition_broadcast(bc[:], rec[:])
                    nc.vector.tensor_mul(
                        xsT2[hp][po:po + 64, qh * 512:(qh + 1) * 512],
                        xs[qh][0:64, :], bc[:])

            # ---- output projection ----
            for qt in range(QT):
                po_ = ps.tile([P, D], F32, tag="proj")
                for hp in range(MC):
                    nc.tensor.matmul(po_[:], xsT2[hp][:, qt * P:(qt + 1) * P],
                                     wo[:, hp, :], start=hp == 0, stop=hp == MC - 1)
                osb = opool.tile([P, D], F32, tag="osb")
                nc.vector.tensor_add(osb[:], po_[:], bo_bc[:])
                nc.sync.dma_start(out_d[qt * P:(qt + 1) * P, :], osb[:])

    nc.compile()
    return nc


def _host_inputs(query, key, value, mask, Wq, bq, Wk, bk, Wv, bv, Wo, bo):
    """Build the 8 per-core input maps (all rank-dependence lives here)."""
    f32 = np.float32
    wq_ = np.ascontiguousarray(Wq).astype(MM_NP)
    wk_ = np.ascontiguousarray(Wk).astype(MM_NP)
    wv_ = np.ascontiguousarray(Wv).astype(MM_NP)
    wo_ = np.ascontiguousarray(Wo).astype(MM_NP)
    bq_ = np.ascontiguousarray(bq.astype(f32).reshape(MC, P).T)
    bk_ = np.ascontiguousarray(bk.astype(f32).reshape(MC, P).T)
    bv_ = bv.astype(MM_NP).reshape(1, D)
    bo_ = bo.astype(f32).reshape(1, D)
    in_maps = []
    for c in range(N_CORES):
        b, half = c // 2, c % 2
        xqT = np.ascontiguousarray(
            query[b, half * LQ:(half + 1) * LQ, :].T).astype(MM_NP)
        xkT = np.ascontiguousarray(key[b].T).astype(MM_NP)
        xvT = np.ascontiguousarray(value[b].T).astype(MM_NP)
        mbias = np.where(mask[b] == 0, MASK_BIAS, f32(0.0)).astype(f32)
        mb_ = np.ascontiguousarray(mbias.reshape(KVC, P).T)
        in_maps.append({
            "xqT": xqT, "xkT": xkT, "xvT": xvT,
            "wq": wq_, "wk": wk_, "wv": wv_, "wo": wo_,
            "bq": bq_, "bk": bk_, "bv": bv_, "bo": bo_, "mb": mb_,
        })
    return in_maps


def kernel(query, key, value, mask, Wq, bq, Wk, bk, Wv, bv, Wo, bo):
    if "nc" not in _cache:
        _cache["nc"] = _build()
    nc = _cache["nc"]
    in_maps = _host_inputs(query, key, value, mask,
                           Wq, bq, Wk, bk, Wv, bv, Wo, bo)
    res = run_bass_kernel_spmd(nc, in_maps, list(range(N_CORES))).results
    out = np.empty((B, L, D), np.float32)
    for c in range(N_CORES):
        b, half = c // 2, c % 2
        out[b, half * LQ:(half + 1) * LQ, :] = res[c]["out"]
    return out
